# revision 1
# baseline (speedup 1.0000x reference)
"""HGNN layer kernel for Trainium2 (8 NeuronCores, Bass/Tile).

out = x @ C_w + C_b + sum_r agg_r,
agg_r[v] = (1/deg_r(v)) * sum_{hyperedges e of rel r with dest v} sum_k x[src_k(e)] @ A_r[k]

Formulation: flatten every (hyperedge, slot) pair into an "incidence"
(src, dest, w=1/deg_r(dest), table t) where table t in {r1s0, r2s0, r2s1,
r3s0, r3s1, r3s2}. The node-linear term x@C_w is table 6 with identity
incidences (src=dest=v, w=1). Then

out[v] = C_b + sum_t ( sum_{i in t: dest_i=v} w_i * x[src_i] ) @ A_t

Device strategy (per core, dest-sharded 12500 nodes = 98 blocks of 128):
  for block b, table t:  P^T[din, slot] = sum_tiles G_tile^T-contract @ S_tile
    - G_tile [128 inc, 128 din] bf16: gathered x rows (indirect DMA)
    - S_tile [128 inc, 128 slot] bf16 = (iota==dest_local)*w  (one DVE op)
    - matmul(lhsT=G, rhs=S) accumulates P^T in PSUM over the (b,t) tiles
  then transform matmul(lhsT=A_t [din,dout], rhs=P^T) accumulates
  U^T[dout, slot] in PSUM over the 7 tables; flush: +C_b (per-partition),
  PE-transpose to [slot, dout], DMA out. No inter-core communication.
"""

import numpy as np
import ml_dtypes

from contextlib import ExitStack

from concourse import bass, bacc, mybir
import concourse.tile as tile
from concourse.bass_utils import run_bass_kernel_spmd

BF16 = ml_dtypes.bfloat16

N_NODES = 100000
D = 128
N_CORES = 8
PER_CORE = N_NODES // N_CORES          # 12500
N_BLK = (PER_CORE + 127) // 128        # 98 (last block 84 rows)
LAST_ROWS = PER_CORE - (N_BLK - 1) * 128  # 84
N_TAB = 7

_cache = {}
LAST_EXEC_NS = None
LAST_PROFILE = None


def _build_incidences(ei_r1, ei_r2, ei_r3):
    """Return (src, dest, w, tab) int32/int32/f32/int8 flat arrays."""
    srcs, dests, ws, tabs = [], [], [], []
    t = 0
    for ei, s in ((ei_r1, 1), (ei_r2, 2), (ei_r3, 3)):
        ei = np.asarray(ei)
        dr = ei[1, ::s].astype(np.int64)
        deg = np.bincount(dr, minlength=N_NODES).astype(np.float32)
        w_e = (1.0 / deg[dr]).astype(np.float32)
        for k in range(s):
            srcs.append(ei[0, k::s].astype(np.int32))
            dests.append(dr.astype(np.int32))
            ws.append(w_e)
            tabs.append(np.full(dr.shape, t, np.int8))
            t += 1
    # table 6: identity (node linear C term)
    ar = np.arange(N_NODES, dtype=np.int32)
    srcs.append(ar)
    dests.append(ar)
    ws.append(np.ones(N_NODES, np.float32))
    tabs.append(np.full(N_NODES, 6, np.int8))
    return (np.concatenate(srcs), np.concatenate(dests),
            np.concatenate(ws), np.concatenate(tabs))


def _host_prep(ei_r1, ei_r2, ei_r3):
    """Bucket incidences by (core, block, table); pad each group to 128.

    Returns n_bt [N_BLK, N_TAB] common tile counts (max over cores) and
    per-core arrays idx[128, T], dest[128, T], w[128, T] (tile-column-major).
    """
    src, dest, w, tab = _build_incidences(ei_r1, ei_r2, ei_r3)
    core = dest // PER_CORE
    loc = dest - core * PER_CORE
    blk = loc >> 7
    slot = loc & 127

    g = ((core.astype(np.int64) * N_BLK + blk) * N_TAB + tab)
    order = np.argsort(g, kind="stable")
    g_s = g[order]
    n_groups = N_CORES * N_BLK * N_TAB
    counts = np.bincount(g_s, minlength=n_groups)
    # per (b,t) tile count = max over cores, >=1
    c3 = counts.reshape(N_CORES, N_BLK, N_TAB)
    n_bt = np.maximum(1, -(-c3.max(axis=0) // 128))        # [N_BLK, N_TAB]
    T_total = int(n_bt.sum())
    col_base = np.zeros((N_BLK, N_TAB), np.int64)
    col_base.ravel()[1:] = np.cumsum(n_bt.ravel())[:-1]

    # rank of each incidence within its group
    group_start = np.zeros(n_groups + 1, np.int64)
    group_start[1:] = np.cumsum(counts)
    rank = np.arange(len(g_s), dtype=np.int64) - group_start[g_s]

    idx_a = np.zeros((N_CORES, 128, T_total), np.int32)
    dst_a = np.zeros((N_CORES, 128, T_total), np.float32)
    w_a = np.zeros((N_CORES, 128, T_total), np.float32)

    core_s = core[order]
    col = col_base[blk[order], tab[order]] + (rank >> 7)
    part = rank & 127
    idx_a[core_s, part, col] = src[order]
    dst_a[core_s, part, col] = slot[order].astype(np.float32)
    w_a[core_s, part, col] = w[order].astype(np.float32)
    return n_bt, col_base, idx_a, dst_a, w_a


def _build_program(n_bt):
    """Emit the SPMD Bass program for common tile counts n_bt."""
    T_total = int(n_bt.sum())
    nc = bacc.Bacc("TRN2", target_bir_lowering=False, debug=False,
                   num_devices=N_CORES)
    f32, bf16, i32 = mybir.dt.float32, mybir.dt.bfloat16, mybir.dt.int32

    x_d = nc.dram_tensor("x_bf", [N_NODES, D], bf16, kind="ExternalInput")
    idx_d = nc.dram_tensor("idx", [128, T_total], i32, kind="ExternalInput")
    dst_d = nc.dram_tensor("mdest", [128, T_total], f32, kind="ExternalInput")
    w_d = nc.dram_tensor("mw", [128, T_total], f32, kind="ExternalInput")
    a_d = nc.dram_tensor("a_all", [D, N_TAB * D], bf16, kind="ExternalInput")
    iota_d = nc.dram_tensor("iota", [128, 128], f32, kind="ExternalInput")
    id_d = nc.dram_tensor("ident", [128, 128], f32, kind="ExternalInput")
    cb_d = nc.dram_tensor("cb", [128, 1], f32, kind="ExternalInput")
    out_d = nc.dram_tensor("out", [PER_CORE, D], f32, kind="ExternalOutput")

    with tile.TileContext(nc) as tc, ExitStack() as ctx:
        gpool = ctx.enter_context(tc.tile_pool(name="g", bufs=12))
        spool = ctx.enter_context(tc.tile_pool(name="s", bufs=12))
        pspool = ctx.enter_context(tc.tile_pool(name="psb", bufs=4))
        uspool = ctx.enter_context(tc.tile_pool(name="usb", bufs=3))
        opool = ctx.enter_context(tc.tile_pool(name="osb", bufs=3))
        psum_p = ctx.enter_context(tc.tile_pool(name="pp", bufs=4, space="PSUM"))
        psum_u = ctx.enter_context(tc.tile_pool(name="pu", bufs=2, space="PSUM"))
        psum_t = ctx.enter_context(tc.tile_pool(name="pt", bufs=2, space="PSUM"))

        idx_sb = nc.alloc_sbuf_tensor("idx_sb", [128, T_total], i32).ap()
        dst_sb = nc.alloc_sbuf_tensor("dst_sb", [128, T_total], f32).ap()
        w_sb = nc.alloc_sbuf_tensor("w_sb", [128, T_total], f32).ap()
        a_sb = nc.alloc_sbuf_tensor("a_sb", [D, N_TAB * D], bf16).ap()
        iota_sb = nc.alloc_sbuf_tensor("iota_sb", [128, 128], f32).ap()
        id_sb = nc.alloc_sbuf_tensor("id_sb", [128, 128], f32).ap()
        cb_sb = nc.alloc_sbuf_tensor("cb_sb", [128, 1], f32).ap()
        nc.sync.dma_start(out=idx_sb[:], in_=idx_d.ap()[:, :])
        nc.sync.dma_start(out=dst_sb[:], in_=dst_d.ap()[:, :])
        nc.sync.dma_start(out=w_sb[:], in_=w_d.ap()[:, :])
        nc.sync.dma_start(out=a_sb[:], in_=a_d.ap()[:, :])
        nc.sync.dma_start(out=iota_sb[:], in_=iota_d.ap()[:, :])
        nc.sync.dma_start(out=id_sb[:], in_=id_d.ap()[:, :])
        nc.sync.dma_start(out=cb_sb[:], in_=cb_d.ap()[:, :])

        col = 0
        for b in range(N_BLK):
            u_ps = psum_u.tile([128, 128], dtype=f32, space="PSUM")
            for t in range(N_TAB):
                nt = int(n_bt[b, t])
                p_ps = psum_p.tile([128, 128], dtype=f32, space="PSUM")
                for j in range(nt):
                    gt = gpool.tile([128, 128], dtype=bf16)
                    nc.gpsimd.indirect_dma_start(
                        out=gt[:], out_offset=None,
                        in_=x_d.ap()[:, :],
                        in_offset=bass.IndirectOffsetOnAxis(
                            ap=idx_sb[:, col:col + 1], axis=0),
                    )
                    st = spool.tile([128, 128], dtype=bf16)
                    nc.vector.tensor_scalar(
                        out=st[:], in0=iota_sb[:],
                        scalar1=dst_sb[:, col:col + 1],
                        scalar2=w_sb[:, col:col + 1],
                        op0=mybir.AluOpType.is_equal,
                        op1=mybir.AluOpType.mult,
                    )
                    nc.tensor.matmul(out=p_ps[:], lhsT=gt[:], rhs=st[:],
                                     start=(j == 0), stop=(j == nt - 1))
                    col += 1
                p_sb = pspool.tile([128, 128], dtype=bf16)
                nc.scalar.copy(out=p_sb[:], in_=p_ps[:])
                nc.tensor.matmul(out=u_ps[:],
                                 lhsT=a_sb[:, t * D:(t + 1) * D],
                                 rhs=p_sb[:],
                                 start=(t == 0), stop=(t == N_TAB - 1))
            u_sb = uspool.tile([128, 128], dtype=f32)
            nc.vector.tensor_scalar(out=u_sb[:], in0=u_ps[:],
                                    scalar1=cb_sb[:, 0:1], scalar2=None,
                                    op0=mybir.AluOpType.add)
            t_ps = psum_t.tile([128, 128], dtype=f32, space="PSUM")
            nc.tensor.transpose(out=t_ps[:], in_=u_sb[:], identity=id_sb[:])
            o_sb = opool.tile([128, 128], dtype=f32)
            nc.scalar.copy(out=o_sb[:], in_=t_ps[:])
            rows = LAST_ROWS if b == N_BLK - 1 else 128
            nc.sync.dma_start(out=out_d.ap()[b * 128:b * 128 + rows, :],
                              in_=o_sb[:rows, :])
    nc.compile()
    return nc


def kernel(x, ei_r1, ei_r2, ei_r3, A_r1, A_r2, A_r3, C_w, C_b):
    global LAST_EXEC_NS, LAST_PROFILE
    import os
    n_bt, col_base, idx_a, dst_a, w_a = _host_prep(ei_r1, ei_r2, ei_r3)
    sig = n_bt.tobytes()
    if sig not in _cache:
        _cache[sig] = _build_program(n_bt)
    nc = _cache[sig]

    x_bf = np.ascontiguousarray(np.asarray(x)).astype(BF16)
    a_all = np.concatenate(
        [np.asarray(A_r1)] +
        [np.asarray(A_r2)[k * D:(k + 1) * D] for k in range(2)] +
        [np.asarray(A_r3)[k * D:(k + 1) * D] for k in range(3)] +
        [np.asarray(C_w)], axis=1).astype(BF16)
    iota = np.ascontiguousarray(np.broadcast_to(
        np.arange(128, dtype=np.float32), (128, 128)))
    ident = np.eye(128, dtype=np.float32)
    cb = np.asarray(C_b).reshape(128, 1).astype(np.float32)

    in_maps = []
    for c in range(N_CORES):
        in_maps.append({
            "x_bf": x_bf, "idx": np.ascontiguousarray(idx_a[c]),
            "mdest": np.ascontiguousarray(dst_a[c]),
            "mw": np.ascontiguousarray(w_a[c]),
            "a_all": a_all, "iota": np.ascontiguousarray(iota),
            "ident": ident, "cb": cb,
        })
    trace = bool(int(os.environ.get("BASS_KERNEL_TRACE", "0")))
    res = run_bass_kernel_spmd(nc, in_maps, list(range(N_CORES)), trace=trace)
    LAST_EXEC_NS = res.exec_time_ns
    LAST_PROFILE = getattr(res, "profile_json", None)
    out = np.concatenate([np.asarray(res.results[c]["out"])
                          for c in range(N_CORES)], axis=0)
    return out.astype(np.float32)

